# revision 12
# baseline (speedup 1.0000x reference)
"""Multi-head causal attention with RoPE on 8 Trainium2 NeuronCores.

Sharding: data-parallel over batch (2 groups of 4 cores) x tensor-parallel
over heads (4 heads / 512 cols of Wq/Wk/Wv per core, 512 rows of Wo).
Each core computes its head-group's Q/K/V projections in transposed layout
([head_dim, seq] -- so no on-device transposes are ever needed), applies
RoPE, runs causal softmax attention (scores kept transposed [tk, tq];
row sums via a ones-vector matmul), and emits its partial output
projection.  The host sums the 4 partials per batch element.

Self-contained: shapes/sharding hardcoded for
  q_input/kv_input [2, 2048, 2048], 16 heads x 128 head_dim.
"""

import math

import numpy as np
import ml_dtypes

B, T, D, H = 2, 2048, 2048, 16
HD = 128          # head dim
HALF = HD // 2    # rope half
P = 128           # partitions
CHUNK = 512       # tq / free-dim chunk
NCORES = 8
GROUPS = 4        # head-groups (tensor-parallel degree per batch)
HPG = H // GROUPS # heads per group
GD = HPG * HD     # group width (512)
DT = D // P       # d-tiles (16)
TCH = T // CHUNK  # seq chunks (4)
TKT = T // P      # tk tiles (16)
CPT = CHUNK // P  # tk tiles per chunk (4)

TRACE = False       # set True before calling kernel() to capture an NTFF trace
TRACE_DIR = None    # optional fixed dir for trace artifacts
LAST_RESULT = None  # BassKernelResults of the last kernel() call
DEBUG_DUMPS = False # add QT/KT/V/AT debug outputs to the program

_cache = {}


def _build_program(actions, npat, pat_off):
    """Build the per-core Bass program (v2 schedule).

    actions: {(c, t): "full" | pattern_index} for every (tq-chunk, tk-tile)
    score block that has at least one unmasked element.

    v2 vs baseline: attention streams t-major across heads with the AV
    matmul LAGged 4 blocks behind its score matmul (so the scalar-engine
    exp never head-of-line-blocks the PE), the per-head row-sum matmuls
    are issued back-to-back into disjoint 32-col groups (concurrent on
    the PE sub-arrays), qproj(c+1)/oproj(c-1) matmul groups are drizzled
    into the attention stream as PE filler, evictions move off the
    scalar engine (DVE copies) during attention, the startup DMAs are
    split across the sync+scalar HWDGE queues, and the output DMA is
    batched per 128-row strip.
    """
    from contextlib import ExitStack

    import concourse.mybir as mybir
    import concourse.tile as tile
    from concourse import bacc
    from concourse.bass import ds, ts

    fp32 = mybir.dt.float32
    bf16 = mybir.dt.bfloat16
    Copy = mybir.ActivationFunctionType.Copy
    Exp = mybir.ActivationFunctionType.Exp
    SCALE = 1.0 / math.sqrt(HD)

    nc = bacc.Bacc(
        "TRN2",
        target_bir_lowering=False,
        debug=False,
        enable_asserts=False,
        num_devices=NCORES,
    )

    xqT = nc.dram_tensor("xqT", [D, T], bf16, kind="ExternalInput").ap()
    xkvT = nc.dram_tensor("xkvT", [D, T], bf16, kind="ExternalInput").ap()
    wq = nc.dram_tensor("wq", [D, GD], bf16, kind="ExternalInput").ap()
    wk = nc.dram_tensor("wk", [D, GD], bf16, kind="ExternalInput").ap()
    wv = nc.dram_tensor("wv", [D, GD], bf16, kind="ExternalInput").ap()
    wo = nc.dram_tensor("wo", [GD, D], bf16, kind="ExternalInput").ap()
    # RoPE in head-dim-interleaved space (host permutes Wq/Wk columns so the
    # rope pair (j, j+64) lands on adjacent partitions (2j, 2j+1); scores are
    # invariant to a common Q/K head-dim permutation):
    #   rope'(x) = x * cs2 + swap_adjacent_pairs(x) * ss2
    # cs2[2j] = cs2[2j+1] = cos_j ; ss2[2j] = -sin_j, ss2[2j+1] = +sin_j
    cs2 = nc.dram_tensor("cs2", [P, T], bf16, kind="ExternalInput").ap()
    ss2 = nc.dram_tensor("ss2", [P, T], bf16, kind="ExternalInput").ap()
    pat = nc.dram_tensor("pat", [npat, P, CHUNK], bf16, kind="ExternalInput").ap()
    out = nc.dram_tensor("out", [T, D], bf16, kind="ExternalOutput").ap()

    with ExitStack() as ctx:
        tc = ctx.enter_context(tile.TileContext(nc))
        const_pool = ctx.enter_context(tc.tile_pool(name="const", bufs=1))
        xpool = ctx.enter_context(tc.tile_pool(name="xchunk", bufs=2))
        rope_pool = ctx.enter_context(tc.tile_pool(name="rope", bufs=2))
        exp_pool = ctx.enter_context(tc.tile_pool(name="exp", bufs=8))
        esum_pool = ctx.enter_context(tc.tile_pool(name="esum", bufs=4))
        osb_pool = ctx.enter_context(tc.tile_pool(name="osb", bufs=2))
        lb_pool = ctx.enter_context(tc.tile_pool(name="lb", bufs=1))
        # PSUM: 8 banks total = a(2) + q(1) + o(4) + l(1).  qproj filler
        # accums get their own 1-buf pool: they are held open across
        # several filler units, and sharing a rotating slot with score
        # tiles would let a score matmul WAR-wait on PE work queued
        # behind it (head-of-line deadlock).
        a_psum = ctx.enter_context(tc.tile_pool(name="aps", bufs=2, space="PSUM"))
        q_psum = ctx.enter_context(tc.tile_pool(name="qps", bufs=1, space="PSUM"))
        o_psum = ctx.enter_context(tc.tile_pool(name="ops", bufs=4, space="PSUM"))
        l_psum = ctx.enter_context(tc.tile_pool(name="lps", bufs=1, space="PSUM"))

        # persistent SBUF tensors
        wq_sb = const_pool.tile([P, DT, GD], bf16, tag="wq")
        wk_sb = const_pool.tile([P, DT, GD], bf16, tag="wk")
        wv_sb = const_pool.tile([P, DT, GD], bf16, tag="wv")
        wo_sb = const_pool.tile([P, HPG, D], bf16, tag="wo")
        cs2_sb = const_pool.tile([P, T], bf16, tag="cs2")
        ss2_sb = const_pool.tile([P, T], bf16, tag="ss2")
        pat_sb = const_pool.tile([P, npat, CHUNK], bf16, tag="pat")
        ones_sb = const_pool.tile([P, 1], bf16, tag="ones")
        QT = const_pool.tile([P, HPG, T], bf16, tag="QT")
        KT = const_pool.tile([P, HPG, T], bf16, tag="KT")
        V = const_pool.tile([P, TKT, GD], bf16, tag="V")
        AT = const_pool.tile([P, HPG, T], bf16, tag="AT")

        wkr = wk.rearrange("(dt p) n -> p dt n", p=P)
        xkvr = xkvT.rearrange("(dt p) t -> p dt t", p=P)
        xqr = xqT.rearrange("(dt p) t -> p dt t", p=P)

        # Startup: wk pieces on the sync queue, xk0 pieces on the scalar
        # queue (both HWDGE) so descriptor generation runs in parallel.
        # Piece sizes ramp [2,2,4,8] d-tiles: the first K-proj matmuls can
        # start after ~0.5MB instead of the full 4MB.
        PIECES = [(0, 2), (2, 2), (4, 4), (8, 8)]
        xk0 = xpool.tile([P, DT, CHUNK], bf16, tag="xc")
        for s0, nd in PIECES:
            nc.sync.dma_start(wk_sb[:, s0:s0 + nd, :], wkr[:, s0:s0 + nd, :])
            nc.scalar.dma_start(
                xk0[:, s0:s0 + nd, :], xkvr[:, s0:s0 + nd, ts(0, CHUNK)]
            )
        nc.scalar.dma_start(cs2_sb[:], cs2)
        nc.scalar.dma_start(ss2_sb[:], ss2)
        nc.sync.dma_start(wv_sb[:], wv.rearrange("(dt p) n -> p dt n", p=P))
        nc.vector.memset(ones_sb[:], 1.0)

        SHUF_MASK = [i + 1 - 2 * (i % 2) for i in range(32)]  # [1,0,3,2,...]
        IDENT32 = list(range(32))
        OCH = min(CHUNK, D)

        def rope_evict(ps, h, c, dest, on_act):
            # ps: PSUM [P, CHUNK] fp32, partitions = interleaved head_dim.
            # The raw copy runs on ACT when the scalar engine is idle (K/V
            # and qproj0 phases) and on DVE when ACT is saturated with exps.
            raw = rope_pool.tile([P, CHUNK], bf16, tag="raw")
            if on_act:
                nc.scalar.activation(raw[:], ps[:], Copy)
            else:
                nc.vector.tensor_copy(raw[:], ps[:])
            rsw = rope_pool.tile([P, CHUNK], bf16, tag="rsw")
            nc.vector.stream_shuffle(rsw[:], raw[:], SHUF_MASK)
            t1 = rope_pool.tile([P, CHUNK], bf16, tag="t1")
            nc.vector.tensor_mul(t1[:], raw[:], cs2_sb[:, ts(c, CHUNK)])
            t2 = rope_pool.tile([P, CHUNK], bf16, tag="t2")
            nc.vector.tensor_mul(t2[:], rsw[:], ss2_sb[:, ts(c, CHUNK)])
            nc.vector.tensor_add(dest[:, h, ts(c, CHUNK)], t1[:], t2[:])

        xq_tiles = {}

        def issue_xq(c, split):
            # xq chunk DMA.  split=True halves across sync+scalar queues
            # (K/V phase: scalar has slack); split=False keeps it off the
            # scalar queue (attention: scalar is the exp bottleneck).
            xq = xpool.tile([P, DT, CHUNK], bf16, tag="xc", name=f"xq{c}")
            if split:
                nc.sync.dma_start(xq[:, 0:8, :], xqr[:, 0:8, ts(c, CHUNK)])
                nc.scalar.dma_start(xq[:, 8:16, :], xqr[:, 8:16, ts(c, CHUNK)])
            else:
                nc.sync.dma_start(xq[:], xqr[:, :, ts(c, CHUNK)])
            xq_tiles[c] = xq

        def qproj_fillers(c, on_act, pool=None, tag="q"):
            # 16 filler units of 4 matmuls each (one PSUM accumulation per
            # head, rope-evicted at the end of its 4th unit).  Interleaved
            # fillers must use the dedicated 1-buf q pool (see pool note);
            # the standalone qproj(0) phase can rotate through the a pool.
            units = []
            accum = {}
            if pool is None:
                pool = q_psum

            def unit(h, u):
                if u == 0:
                    accum[h] = pool.tile([P, CHUNK], fp32, tag=tag, name="qacc")
                ps = accum[h]
                for d in range(4 * u, 4 * u + 4):
                    nc.tensor.matmul(
                        ps[:], wq_sb[:, d, ts(h, HD)], xq_tiles[c][:, d, :],
                        start=(d == 0), stop=(d == DT - 1),
                    )
                if u == 3:
                    rope_evict(ps, h, c, QT, on_act)

            for h in range(HPG):
                for u in range(4):
                    units.append(lambda h=h, u=u: unit(h, u))
            return units

        def oproj_fillers(co):
            # 16 filler units of 4 matmuls each: out[tq,:] partial rows for
            # chunk co.  DVE evicts into a [P, D] staging strip; one batched
            # DMA per 128-row strip (4KB contiguous lines).
            units = []
            obs = {}

            def unit(m, oc):
                if oc == 0:
                    obs[m] = osb_pool.tile([P, D], bf16, tag="ob", name="ob")
                ps = a_psum.tile([P, OCH], fp32, tag="a", name="ops2")
                for h in range(HPG):
                    nc.tensor.matmul(
                        ps[:], AT[:, h, ts(m, P)], wo_sb[:, h, ts(oc, OCH)],
                        start=(h == 0), stop=(h == HPG - 1),
                    )
                nc.vector.tensor_copy(obs[m][:, ts(oc, OCH)], ps[:])
                if oc == D // OCH - 1:
                    nc.sync.dma_start(out[ts(m, P), :], obs[m][:])

            for m in range(4 * co, 4 * co + 4):
                for oc in range(D // OCH):
                    units.append(lambda m=m, oc=oc: unit(m, oc))
            return units

        def attn_chunk(c, fillers):
            # t-major across heads; AV lags its score matmul by LAG blocks
            # so the ACT exp sits between two PE ops that are never
            # adjacent in the PE queue.  Row-sum matmuls are emitted 4
            # back-to-back into col-groups {0,32,64,96} of one PSUM bank
            # (concurrent sub-array execution).  Fillers pop between
            # blocks at a uniform rate.
            tlist = [t for t in range(TKT) if (c, t) in actions]
            full_ts = [t for t in tlist if actions[(c, t)] == "full"]
            diag_ts = [t for t in tlist if actions[(c, t)] != "full"]
            order_ts = full_ts + diag_ts
            quads = [full_ts[i:i + 4] for i in range(0, len(full_ts), 4)]
            n_lgrp = len(quads) + len(diag_ts)
            qinfo = {}
            for quad in quads:
                for j, t in enumerate(quad):
                    qinfo[t] = (j, len(quad))
            blocks = [(t, h) for t in order_ts for h in range(HPG)]
            nb = len(blocks)
            LAG = 4

            lpst = l_psum.tile([P, CHUNK], fp32, tag="l")
            opst = [o_psum.tile([P, CHUNK], fp32, tag="o", name=f"o{h}")
                    for h in range(HPG)]
            es_store = {}
            esq = {}
            esq_first = {}
            lgrp = [0]
            av_cnt = [0] * HPG
            nav = len(tlist)

            def emit_s(t, h):
                a = actions[(c, t)]
                off = 0 if a == "full" else pat_off[a]
                w = CHUNK - off
                sp = a_psum.tile([P, CHUNK], fp32, tag="a", name="sps")
                nc.tensor.matmul(
                    sp[:, ds(off, w)], KT[:, h, ts(t, P)],
                    QT[:, h, ds(c * CHUNK + off, w)],
                    start=True, stop=True,
                )
                es = exp_pool.tile([P, CHUNK], bf16, tag="es")
                nc.scalar.activation(es[:, ds(off, w)], sp[:, ds(off, w)],
                                     Exp, scale=SCALE)
                if a != "full":
                    nc.vector.tensor_mul(
                        es[:, ds(off, w)], es[:, ds(off, w)],
                        pat_sb[:, a, ds(off, w)],
                    )
                es_store[(t, h)] = (es, off, w)

            def lmm(hh, src, off, w):
                r = 32 * hh
                nc.tensor.matmul(
                    lpst[r:r + 1, ds(off, w)], ones_sb[:], src[:, ds(off, w)],
                    start=(lgrp[0] == 0), stop=(lgrp[0] == n_lgrp - 1),
                    tile_position=(0, r),
                )

            def emit_av(t, h):
                es, off, w = es_store[(t, h)]
                nc.tensor.matmul(
                    opst[h][:, ds(off, w)], V[:, t, ts(h, HD)], es[:, ds(off, w)],
                    start=(av_cnt[h] == 0), stop=(av_cnt[h] == nav - 1),
                )
                av_cnt[h] += 1
                if actions[(c, t)] == "full":
                    j, qlen = qinfo[t]
                    if qlen == 1:
                        esq[h] = es
                    elif j == 0:
                        esq_first[h] = es
                    elif j == 1:
                        tmp = esum_pool.tile([P, CHUNK], bf16, tag="esq")
                        nc.vector.tensor_add(tmp[:], esq_first[h][:], es[:])
                        esq[h] = tmp
                    else:
                        nc.vector.tensor_add(esq[h][:], esq[h][:], es[:])
                    if j == qlen - 1 and h == HPG - 1:
                        for hh in range(HPG):
                            lmm(hh, esq[hh], 0, CHUNK)
                        lgrp[0] += 1
                else:
                    if h == HPG - 1:
                        for hh in range(HPG):
                            ess, off2, w2 = es_store[(t, hh)]
                            lmm(hh, ess, off2, w2)
                        lgrp[0] += 1

            fill_acc = 0.0
            fill_rate = len(fillers) / nb if nb else 0.0
            for i in range(nb + LAG):
                if i >= LAG:
                    emit_av(*blocks[i - LAG])
                fill_acc += fill_rate
                while fill_acc >= 1.0 and fillers:
                    fillers.pop(0)()
                    fill_acc -= 1.0
                if i < nb:
                    emit_s(*blocks[i])

            # unnormalized O^T eviction (DVE), then the 1/l normalize chain
            for h in range(HPG):
                nc.vector.tensor_copy(AT[:, h, ts(c, CHUNK)], opst[h][:])
            rec = lb_pool.tile([P, CHUNK], fp32, tag="rec")
            nc.vector.reciprocal(rec[:], lpst[:])
            for h in range(HPG):
                r = 32 * h
                if h == 0:
                    src = rec
                else:
                    src = lb_pool.tile([32, CHUNK], fp32, tag="shf")
                    nc.vector.stream_shuffle(src[:], rec[r:r + 32, :], IDENT32)
                lbs = lb_pool.tile([P, CHUNK], fp32, tag="lbs")
                nc.gpsimd.partition_broadcast(lbs[:], src[0:1, :])
                nc.vector.tensor_mul(
                    AT[:, h, ts(c, CHUNK)], AT[:, h, ts(c, CHUNK)], lbs[:]
                )

        # ---- K^T / V projections over all chunks ----
        xk_tiles = {0: xk0}
        for c in range(TCH):
            xk = xk_tiles[c]
            if c + 1 < TCH:
                nxt = xpool.tile([P, DT, CHUNK], bf16, tag="xc", name=f"xk{c + 1}")
                nc.sync.dma_start(nxt[:, 0:8, :], xkvr[:, 0:8, ts(c + 1, CHUNK)])
                nc.scalar.dma_start(nxt[:, 8:16, :], xkvr[:, 8:16, ts(c + 1, CHUNK)])
                xk_tiles[c + 1] = nxt
            for h in range(HPG):
                ps = a_psum.tile([P, CHUNK], fp32, tag="a", name="kps")
                for d in range(DT):
                    nc.tensor.matmul(
                        ps[:], wk_sb[:, d, ts(h, HD)], xk[:, d, :],
                        start=(d == 0), stop=(d == DT - 1),
                    )
                rope_evict(ps, h, c, KT, on_act=True)
            for s in range(CPT):
                ps = a_psum.tile([P, GD], fp32, tag="a", name="vps")
                for d in range(DT):
                    nc.tensor.matmul(
                        ps[:], xk[:, d, ts(s, P)], wv_sb[:, d, :],
                        start=(d == 0), stop=(d == DT - 1),
                    )
                nc.scalar.activation(V[:, c * CPT + s, :], ps[:], Copy)
            if c == 0:
                nc.sync.dma_start(wq_sb[:], wq.rearrange("(dt p) n -> p dt n", p=P))
                nc.sync.dma_start(pat_sb[:], pat.rearrange("j p n -> p j n"))
            elif c == 1:
                nc.sync.dma_start(wo_sb[:], wo.rearrange("(h p) n -> p h n", p=P))
            elif c == 2:
                issue_xq(0, split=True)
            elif c == 3:
                issue_xq(1, split=True)

        # ---- qproj(0) standalone, then attention with filler drizzle ----
        for u in qproj_fillers(0, on_act=True, pool=a_psum, tag="a"):
            u()
        fillq = []
        for c in range(TCH):
            if c == 0:
                issue_xq(2, split=False)
            elif c == 1:
                issue_xq(3, split=False)
            if c + 1 < TCH:
                fillq.extend(qproj_fillers(c + 1, on_act=False))
            if c >= 1:
                fillq.extend(oproj_fillers(c - 1))
            attn_chunk(c, fillq)
        for u in fillq:
            u()
        for u in oproj_fillers(TCH - 1):
            u()

        if DEBUG_DUMPS:
            for nm, sb in [("dQT", QT), ("dKT", KT), ("dV", V), ("dAT", AT)]:
                dt_ = nc.dram_tensor(nm, list(sb.shape), bf16, kind="ExternalOutput").ap()
                nc.sync.dma_start(dt_[:], sb[:])

    nc.compile()
    return nc


def _interleave_heads(W):
    """Permute each 128-wide head block of columns: new[2j]=old[j], new[2j+1]=old[64+j]."""
    d, gd = W.shape
    return np.ascontiguousarray(
        W.reshape(d, gd // HD, 2, HALF).transpose(0, 1, 3, 2).reshape(d, gd)
    )


def _rope_tables(cos, sin):
    """cs2[2j]=cs2[2j+1]=cos_j ; ss2[2j]=-sin_j, ss2[2j+1]=+sin_j  (both [128, T])."""
    bf = ml_dtypes.bfloat16
    cosT = np.ascontiguousarray(cos.T)  # [HALF, T]
    sinT = np.ascontiguousarray(sin.T)
    cs2 = np.repeat(cosT, 2, axis=0).astype(bf)
    ss2 = np.stack([-sinT, sinT], axis=1).reshape(HD, -1).astype(bf)
    return cs2, ss2


def _mask_actions(mask):
    """Classify every [CHUNK tq x P tk] score block of the mask.

    Returns ({(c, t): "full" | pattern_idx}, patterns [npat, P, CHUNK] bf16).
    Blocks with no unmasked element are omitted (skipped entirely).
    Patterns are stored transposed ([tk, tq]) to match the score layout.
    """
    m = np.asarray(mask).reshape(T, T).astype(bool)
    actions = {}
    pats = []
    pat_keys = {}
    pat_off = {}
    for c in range(TCH):
        for t in range(TKT):
            blk = m[c * CHUNK : (c + 1) * CHUNK, t * P : (t + 1) * P]
            if not blk.any():
                continue
            if blk.all():
                actions[(c, t)] = "full"
                continue
            bt = np.ascontiguousarray(blk.T)
            key = bt.tobytes()
            if key not in pat_keys:
                pat_keys[key] = len(pats)
                # first tq column with any valid element: ops on this
                # pattern only touch [off, CHUNK)
                pat_off[len(pats)] = int(np.argmax(bt.any(axis=0)))
                pats.append(bt.astype(ml_dtypes.bfloat16))
            actions[(c, t)] = pat_keys[key]
    if not pats:
        pats.append(np.zeros((P, CHUNK), ml_dtypes.bfloat16))
    return actions, np.ascontiguousarray(np.stack(pats)), pat_off


def kernel(**inputs):
    global LAST_RESULT
    q_input = np.asarray(inputs["q_input"], dtype=np.float32)
    kv_input = np.asarray(inputs["kv_input"], dtype=np.float32)
    cos = np.asarray(inputs["cos"], dtype=np.float32)
    sin = np.asarray(inputs["sin"], dtype=np.float32)
    Wq = np.asarray(inputs["Wq"], dtype=np.float32)
    Wk = np.asarray(inputs["Wk"], dtype=np.float32)
    Wv = np.asarray(inputs["Wv"], dtype=np.float32)
    Wo = np.asarray(inputs["Wo"], dtype=np.float32)

    actions, pats, pat_off = _mask_actions(inputs["mask"])
    key = (tuple(sorted(actions.items())), pats.shape[0], tuple(sorted(pat_off.items())))
    if key not in _cache:
        _cache[key] = _build_program(actions, int(pats.shape[0]), pat_off)
    nc = _cache[key]

    bf = ml_dtypes.bfloat16
    cs2, ss2 = _rope_tables(cos, sin)
    xq = [np.ascontiguousarray(q_input[b].T).astype(bf) for b in range(B)]
    xkv = [np.ascontiguousarray(kv_input[b].T).astype(bf) for b in range(B)]
    wq_g = [_interleave_heads(Wq[:, g * GD : (g + 1) * GD]).astype(bf) for g in range(GROUPS)]
    wk_g = [_interleave_heads(Wk[:, g * GD : (g + 1) * GD]).astype(bf) for g in range(GROUPS)]
    wv_g = [np.ascontiguousarray(Wv[:, g * GD : (g + 1) * GD]).astype(bf) for g in range(GROUPS)]
    wo_g = [np.ascontiguousarray(Wo[g * GD : (g + 1) * GD, :]).astype(bf) for g in range(GROUPS)]

    in_maps = []
    for core in range(NCORES):
        b, g = divmod(core, GROUPS)
        in_maps.append({
            "xqT": xq[b],
            "xkvT": xkv[b],
            "wq": wq_g[g],
            "wk": wk_g[g],
            "wv": wv_g[g],
            "wo": wo_g[g],
            "cs2": cs2,
            "ss2": ss2,
            "pat": pats,
        })

    from concourse import bass_utils

    res = bass_utils.run_bass_kernel_spmd(
        nc, in_maps, core_ids=list(range(NCORES)), trace=TRACE, tmpdir=TRACE_DIR
    )
    LAST_RESULT = res
    outs = [np.asarray(r["out"], dtype=np.float32) for r in res.results]
    full = np.stack(
        [sum(outs[b * GROUPS + g] for g in range(GROUPS)) for b in range(B)]
    )
    return np.ascontiguousarray(full.astype(np.float32))



# revision 21
# speedup vs baseline: 1.0253x; 1.0253x over previous
"""Multi-head causal attention with RoPE on 8 Trainium2 NeuronCores.

Sharding: data-parallel over batch (2 groups of 4 cores) x tensor-parallel
over heads (4 heads / 512 cols of Wq/Wk/Wv per core, 512 rows of Wo).
Each core computes its head-group's Q/K/V projections in transposed layout
([head_dim, seq] -- so no on-device transposes are ever needed), applies
RoPE, runs causal softmax attention (scores kept transposed [tk, tq];
row sums via a ones-vector matmul), and emits its partial output
projection.  The host sums the 4 partials per batch element.

Self-contained: shapes/sharding hardcoded for
  q_input/kv_input [2, 2048, 2048], 16 heads x 128 head_dim.
"""

import math

import numpy as np
import ml_dtypes

B, T, D, H = 2, 2048, 2048, 16
HD = 128          # head dim
HALF = HD // 2    # rope half
P = 128           # partitions
CHUNK = 512       # tq / free-dim chunk
NCORES = 8
GROUPS = 4        # head-groups (tensor-parallel degree per batch)
HPG = H // GROUPS # heads per group
GD = HPG * HD     # group width (512)
DT = D // P       # d-tiles (16)
TCH = T // CHUNK  # seq chunks (4)
TKT = T // P      # tk tiles (16)
CPT = CHUNK // P  # tk tiles per chunk (4)

TRACE = False       # set True before calling kernel() to capture an NTFF trace
TRACE_DIR = None    # optional fixed dir for trace artifacts
LAST_RESULT = None  # BassKernelResults of the last kernel() call
DEBUG_DUMPS = False # add QT/KT/V/AT debug outputs to the program

_cache = {}


def _build_program(actions, npat, pat_off):
    """Build the per-core Bass program (v2 schedule).

    actions: {(c, t): "full" | pattern_index} for every (tq-chunk, tk-tile)
    score block that has at least one unmasked element.

    v2 vs baseline: attention streams t-major across heads with the AV
    matmul LAGged 4 blocks behind its score matmul (so the scalar-engine
    exp never head-of-line-blocks the PE), the per-head row-sum matmuls
    are issued back-to-back into disjoint 32-col groups (concurrent on
    the PE sub-arrays), qproj(c+1)/oproj(c-1) matmul groups are drizzled
    into the attention stream as PE filler, evictions move off the
    scalar engine (DVE copies) during attention, the startup DMAs are
    split across the sync+scalar HWDGE queues, and the output DMA is
    batched per 128-row strip.
    """
    from contextlib import ExitStack

    import concourse.mybir as mybir
    import concourse.tile as tile
    from concourse import bacc
    from concourse.bass import ds, ts

    fp32 = mybir.dt.float32
    bf16 = mybir.dt.bfloat16
    Copy = mybir.ActivationFunctionType.Copy
    Exp = mybir.ActivationFunctionType.Exp
    SCALE = 1.0 / math.sqrt(HD)

    nc = bacc.Bacc(
        "TRN2",
        target_bir_lowering=False,
        debug=False,
        enable_asserts=False,
        num_devices=NCORES,
    )

    # All large inputs arrive HOST-PACKED in their exact SBUF layouts
    # (partition-major), so every DMA is one long contiguous run per
    # partition (16KB lines) instead of thousands of 1KB descriptors.
    xqp = nc.dram_tensor("xqp", [P, TCH, DT, CHUNK], bf16, kind="ExternalInput").ap()
    xkvp = nc.dram_tensor("xkvp", [P, TCH, DT, CHUNK], bf16, kind="ExternalInput").ap()
    wq = nc.dram_tensor("wq", [P, DT, GD], bf16, kind="ExternalInput").ap()
    wk = nc.dram_tensor("wk", [P, DT, GD], bf16, kind="ExternalInput").ap()
    wv = nc.dram_tensor("wv", [P, DT, GD], bf16, kind="ExternalInput").ap()
    wo = nc.dram_tensor("wo", [P, HPG, D], bf16, kind="ExternalInput").ap()
    # RoPE in head-dim-interleaved space (host permutes Wq/Wk columns so the
    # rope pair (j, j+64) lands on adjacent partitions (2j, 2j+1); scores are
    # invariant to a common Q/K head-dim permutation):
    #   rope'(x) = x * cs2 + swap_adjacent_pairs(x) * ss2
    # cs2[2j] = cs2[2j+1] = cos_j ; ss2[2j] = -sin_j, ss2[2j+1] = +sin_j
    cs2 = nc.dram_tensor("cs2", [P, T], bf16, kind="ExternalInput").ap()
    ss2 = nc.dram_tensor("ss2", [P, T], bf16, kind="ExternalInput").ap()
    pat = nc.dram_tensor("pat", [npat, P, CHUNK], bf16, kind="ExternalInput").ap()
    out = nc.dram_tensor("out", [T, D], bf16, kind="ExternalOutput").ap()

    with ExitStack() as ctx:
        tc = ctx.enter_context(tile.TileContext(nc))
        const_pool = ctx.enter_context(tc.tile_pool(name="const", bufs=1))
        xpool = ctx.enter_context(tc.tile_pool(name="xchunk", bufs=2))
        rope_pool = ctx.enter_context(tc.tile_pool(name="rope", bufs=2))
        exp_pool = ctx.enter_context(tc.tile_pool(name="exp", bufs=9))
        osb_pool = ctx.enter_context(tc.tile_pool(name="osb", bufs=2))
        lb_pool = ctx.enter_context(tc.tile_pool(name="lb", bufs=1))
        # PSUM: 8 banks total = a(2) + q(1) + o(4) + l(1).  qproj filler
        # accums get their own 1-buf pool: they are held open across
        # several filler units, and sharing a rotating slot with score
        # tiles would let a score matmul WAR-wait on PE work queued
        # behind it (head-of-line deadlock).
        a_psum = ctx.enter_context(tc.tile_pool(name="aps", bufs=2, space="PSUM"))
        q_psum = ctx.enter_context(tc.tile_pool(name="qps", bufs=1, space="PSUM"))
        o_psum = ctx.enter_context(tc.tile_pool(name="ops", bufs=4, space="PSUM"))
        l_psum = ctx.enter_context(tc.tile_pool(name="lps", bufs=1, space="PSUM"))

        # persistent SBUF tensors
        wq_sb = const_pool.tile([P, DT, GD], bf16, tag="wq")
        wk_sb = const_pool.tile([P, DT, GD], bf16, tag="wk")
        wv_sb = const_pool.tile([P, DT, GD], bf16, tag="wv")
        wo_sb = const_pool.tile([P, HPG, D], bf16, tag="wo")
        cs2_sb = const_pool.tile([P, T], bf16, tag="cs2")
        ss2_sb = const_pool.tile([P, T], bf16, tag="ss2")
        pat_sb = const_pool.tile([P, npat, CHUNK], bf16, tag="pat")
        ones_sb = const_pool.tile([P, 1], bf16, tag="ones")
        QT = const_pool.tile([P, HPG, T], bf16, tag="QT")
        KT = const_pool.tile([P, HPG, T], bf16, tag="KT")
        V = const_pool.tile([P, TKT, GD], bf16, tag="V")
        AT = const_pool.tile([P, HPG, T], bf16, tag="AT")

        # Startup: packed layouts make each dma_start ~128 descriptors, so
        # issue cost is negligible.  Piece sizes ramp [2,2,4,8] d-tiles:
        # the first K-proj matmuls start after ~0.5MB instead of 4MB.
        # cs2/ss2 ride the scalar HWDGE queue (idle at startup).
        PIECES = [(0, 2), (2, 2), (4, 4), (8, 8)]
        xk0 = xpool.tile([P, DT, CHUNK], bf16, tag="xc")
        for s0, nd in PIECES:
            nc.sync.dma_start(wk_sb[:, s0:s0 + nd, :], wk[:, s0:s0 + nd, :])
            nc.sync.dma_start(xk0[:, s0:s0 + nd, :], xkvp[:, 0, s0:s0 + nd, :])
        nc.scalar.dma_start(cs2_sb[:], cs2)
        nc.scalar.dma_start(ss2_sb[:], ss2)
        nc.sync.dma_start(wv_sb[:], wv[:])
        nc.vector.memset(ones_sb[:], 1.0)

        SHUF_MASK = [i + 1 - 2 * (i % 2) for i in range(32)]  # [1,0,3,2,...]
        IDENT32 = list(range(32))
        OCH = min(CHUNK, D)

        def rope_evict(ps, h, c, dest):
            # ps: PSUM [P, CHUNK] fp32, partitions = interleaved head_dim.
            # Raw copy always on ACT: a DVE fp32->bf16 copy lowers to a 1x
            # CAST (~690ns) so there is nothing to save on DVE, and the ACT
            # copy frees the PSUM slot through a much shallower queue.
            raw = rope_pool.tile([P, CHUNK], bf16, tag="raw")
            nc.scalar.activation(raw[:], ps[:], Copy)
            rsw = rope_pool.tile([P, CHUNK], bf16, tag="rsw")
            nc.vector.stream_shuffle(rsw[:], raw[:], SHUF_MASK)
            t1 = rope_pool.tile([P, CHUNK], bf16, tag="t1")
            nc.vector.tensor_mul(t1[:], raw[:], cs2_sb[:, ts(c, CHUNK)])
            t2 = rope_pool.tile([P, CHUNK], bf16, tag="t2")
            nc.vector.tensor_mul(t2[:], rsw[:], ss2_sb[:, ts(c, CHUNK)])
            nc.vector.tensor_add(dest[:, h, ts(c, CHUNK)], t1[:], t2[:])

        xq_tiles = {}

        def issue_xq(c):
            xq = xpool.tile([P, DT, CHUNK], bf16, tag="xc", name=f"xq{c}")
            nc.sync.dma_start(xq[:], xqp[:, c, :, :])
            xq_tiles[c] = xq

        def qproj_fillers(c, pool=None, tag="q"):
            # 16 filler units of 4 matmuls each (one PSUM accumulation per
            # head, rope-evicted at the end of its 4th unit).  Interleaved
            # fillers use the 1-buf q pool; qproj(0) standalone rotates
            # through the a pool.  The fillq order (all of one accum's
            # units before anything else touches the q pool) is what makes
            # the 1-buf sharing deadlock-free.
            units = []
            accum = {}
            if pool is None:
                pool = q_psum

            def unit(h, u):
                if u == 0:
                    accum[h] = pool.tile([P, CHUNK], fp32, tag=tag, name="qacc")
                ps = accum[h]
                for d in range(4 * u, 4 * u + 4):
                    nc.tensor.matmul(
                        ps[:], wq_sb[:, d, ts(h, HD)], xq_tiles[c][:, d, :],
                        start=(d == 0), stop=(d == DT - 1),
                    )
                if u == 3:
                    rope_evict(ps, h, c, QT)

            for h in range(HPG):
                for u in range(4):
                    units.append(lambda h=h, u=u: unit(h, u))
            return units

        def oproj_fillers(co):
            # 16 filler units of 4 matmuls each: out[tq,:] partial rows for
            # chunk co.  PSUM from the q pool (never the score pool -- a
            # lagging eviction must throttle only the filler stream, not
            # the attention scores).  DVE evicts into a [P, D] staging
            # strip; one batched DMA per 128-row strip (4KB lines).
            units = []
            obs = {}

            def unit(m, oc):
                if oc == 0:
                    obs[m] = osb_pool.tile([P, D], bf16, tag="ob", name="ob")
                ps = q_psum.tile([P, OCH], fp32, tag="q", name="ops2")
                for h in range(HPG):
                    nc.tensor.matmul(
                        ps[:], AT[:, h, ts(m, P)], wo_sb[:, h, ts(oc, OCH)],
                        start=(h == 0), stop=(h == HPG - 1),
                    )
                nc.vector.tensor_copy(obs[m][:, ts(oc, OCH)], ps[:])
                if oc == D // OCH - 1:
                    nc.sync.dma_start(out[ts(m, P), :], obs[m][:])

            for m in range(4 * co, 4 * co + 4):
                for oc in range(D // OCH):
                    units.append(lambda m=m, oc=oc: unit(m, oc))
            return units

        def attn_chunk(c, fillers, max_pops=10 ** 9):
            # t-major across heads; AV lags its score matmul by LAG blocks
            # so the ACT exp sits between two PE ops that are never
            # adjacent in the PE queue.  Row-sum matmuls are emitted 4
            # back-to-back into col-groups {0,32,64,96} of one PSUM bank
            # (concurrent sub-array execution), one group per t directly
            # on the es tiles (no DVE pre-summing).  Fillers pop between
            # blocks at a uniform rate, capped at max_pops for this chunk.
            tlist = [t for t in range(TKT) if (c, t) in actions]
            full_ts = [t for t in tlist if actions[(c, t)] == "full"]
            diag_ts = [t for t in tlist if actions[(c, t)] != "full"]
            order_ts = full_ts + diag_ts
            n_lgrp = len(tlist)
            blocks = [(t, h) for t in order_ts for h in range(HPG)]
            nb = len(blocks)
            LAG = 5

            lpst = l_psum.tile([P, CHUNK], fp32, tag="l")
            opst = [o_psum.tile([P, CHUNK], fp32, tag="o", name=f"o{h}")
                    for h in range(HPG)]
            es_store = {}
            lgrp = [0]
            av_cnt = [0] * HPG
            nav = len(tlist)

            def emit_s(t, h):
                a = actions[(c, t)]
                off = 0 if a == "full" else pat_off[a]
                w = CHUNK - off
                sp = a_psum.tile([P, CHUNK], fp32, tag="a", name="sps")
                nc.tensor.matmul(
                    sp[:, ds(off, w)], KT[:, h, ts(t, P)],
                    QT[:, h, ds(c * CHUNK + off, w)],
                    start=True, stop=True,
                )
                es = exp_pool.tile([P, CHUNK], bf16, tag="es")
                nc.scalar.activation(es[:, ds(off, w)], sp[:, ds(off, w)],
                                     Exp, scale=SCALE)
                if a != "full":
                    nc.vector.tensor_mul(
                        es[:, ds(off, w)], es[:, ds(off, w)],
                        pat_sb[:, a, ds(off, w)],
                    )
                es_store[(t, h)] = (es, off, w)

            def lmm(hh, src, off, w):
                r = 32 * hh
                nc.tensor.matmul(
                    lpst[r:r + 1, ds(off, w)], ones_sb[:], src[:, ds(off, w)],
                    start=(lgrp[0] == 0), stop=(lgrp[0] == n_lgrp - 1),
                    tile_position=(0, r),
                )

            def emit_av(t, h):
                es, off, w = es_store[(t, h)]
                nc.tensor.matmul(
                    opst[h][:, ds(off, w)], V[:, t, ts(h, HD)], es[:, ds(off, w)],
                    start=(av_cnt[h] == 0), stop=(av_cnt[h] == nav - 1),
                )
                av_cnt[h] += 1
                if h == HPG - 1:
                    for hh in range(HPG):
                        ess, off2, w2 = es_store[(t, hh)]
                        lmm(hh, ess, off2, w2)
                    lgrp[0] += 1

            fill_acc = 0.0
            pops = [0]
            navail = min(len(fillers), max_pops)
            fill_rate = navail / nb if nb else 0.0
            for i in range(nb + LAG):
                if i >= LAG:
                    emit_av(*blocks[i - LAG])
                fill_acc += fill_rate
                while fill_acc >= 1.0 and fillers and pops[0] < max_pops:
                    fillers.pop(0)()
                    pops[0] += 1
                    fill_acc -= 1.0
                if i < nb:
                    emit_s(*blocks[i])

            # unnormalized O^T eviction (ACT: o-banks free through a
            # shallow queue at the chunk boundary), then the 1/l chain
            for h in range(HPG):
                nc.scalar.activation(AT[:, h, ts(c, CHUNK)], opst[h][:], Copy)
            rec = lb_pool.tile([P, CHUNK], fp32, tag="rec")
            nc.vector.reciprocal_approx_fast(rec[:], lpst[:])
            for h in range(HPG):
                r = 32 * h
                if h == 0:
                    src = rec
                else:
                    src = lb_pool.tile([32, CHUNK], fp32, tag="shf")
                    nc.vector.stream_shuffle(src[:], rec[r:r + 32, :], IDENT32)
                lbs = lb_pool.tile([P, CHUNK], fp32, tag="lbs")
                nc.gpsimd.partition_broadcast(lbs[:], src[0:1, :])
                nc.vector.tensor_mul(
                    AT[:, h, ts(c, CHUNK)], AT[:, h, ts(c, CHUNK)], lbs[:]
                )

        # ---- K^T / V projections over all chunks ----
        xk_tiles = {0: xk0}
        for c in range(TCH):
            xk = xk_tiles[c]
            if c + 1 < TCH:
                nxt = xpool.tile([P, DT, CHUNK], bf16, tag="xc", name=f"xk{c + 1}")
                nc.sync.dma_start(nxt[:], xkvp[:, c + 1, :, :])
                xk_tiles[c + 1] = nxt
            for h in range(HPG):
                ps = a_psum.tile([P, CHUNK], fp32, tag="a", name="kps")
                for d in range(DT):
                    nc.tensor.matmul(
                        ps[:], wk_sb[:, d, ts(h, HD)], xk[:, d, :],
                        start=(d == 0), stop=(d == DT - 1),
                    )
                rope_evict(ps, h, c, KT)
            for s in range(CPT):
                ps = a_psum.tile([P, GD], fp32, tag="a", name="vps")
                for d in range(DT):
                    nc.tensor.matmul(
                        ps[:], xk[:, d, ts(s, P)], wv_sb[:, d, :],
                        start=(d == 0), stop=(d == DT - 1),
                    )
                nc.scalar.activation(V[:, c * CPT + s, :], ps[:], Copy)
            if c == 0:
                nc.sync.dma_start(wq_sb[:], wq[:])
                nc.sync.dma_start(pat_sb[:], pat.rearrange("j p n -> p j n"))
            elif c == 1:
                nc.sync.dma_start(wo_sb[:], wo[:])
            elif c == 2:
                issue_xq(0)
            elif c == 3:
                issue_xq(1)

        # ---- qproj(0) standalone, then attention with filler drizzle ----
        # Filler budgets shift oproj work out of the PE-bound middle
        # chunks into the ACT-bound final chunk (and its tail).
        for u in qproj_fillers(0, pool=a_psum, tag="a"):
            u()
        BUDGET = {0: 10 ** 9, 1: 10 ** 9, 2: 26, 3: 10 ** 9}
        fillq = []
        for c in range(TCH):
            if c == 0:
                issue_xq(2)
            elif c == 1:
                issue_xq(3)
            if c + 1 < TCH:
                fillq.extend(qproj_fillers(c + 1))
            if c >= 1:
                fillq.extend(oproj_fillers(c - 1))
            attn_chunk(c, fillq, BUDGET[c])
        for u in fillq:
            u()
        for u in oproj_fillers(TCH - 1):
            u()

        if DEBUG_DUMPS:
            for nm, sb in [("dQT", QT), ("dKT", KT), ("dV", V), ("dAT", AT)]:
                dt_ = nc.dram_tensor(nm, list(sb.shape), bf16, kind="ExternalOutput").ap()
                nc.sync.dma_start(dt_[:], sb[:])

    nc.compile()
    return nc


def _interleave_heads(W):
    """Permute each 128-wide head block of columns: new[2j]=old[j], new[2j+1]=old[64+j]."""
    d, gd = W.shape
    return np.ascontiguousarray(
        W.reshape(d, gd // HD, 2, HALF).transpose(0, 1, 3, 2).reshape(d, gd)
    )


def _rope_tables(cos, sin):
    """cs2[2j]=cs2[2j+1]=cos_j ; ss2[2j]=-sin_j, ss2[2j+1]=+sin_j  (both [128, T])."""
    bf = ml_dtypes.bfloat16
    cosT = np.ascontiguousarray(cos.T)  # [HALF, T]
    sinT = np.ascontiguousarray(sin.T)
    cs2 = np.repeat(cosT, 2, axis=0).astype(bf)
    ss2 = np.stack([-sinT, sinT], axis=1).reshape(HD, -1).astype(bf)
    return cs2, ss2


def _mask_actions(mask):
    """Classify every [CHUNK tq x P tk] score block of the mask.

    Returns ({(c, t): "full" | pattern_idx}, patterns [npat, P, CHUNK] bf16).
    Blocks with no unmasked element are omitted (skipped entirely).
    Patterns are stored transposed ([tk, tq]) to match the score layout.
    """
    m = np.asarray(mask).reshape(T, T).astype(bool)
    actions = {}
    pats = []
    pat_keys = {}
    pat_off = {}
    for c in range(TCH):
        for t in range(TKT):
            blk = m[c * CHUNK : (c + 1) * CHUNK, t * P : (t + 1) * P]
            if not blk.any():
                continue
            if blk.all():
                actions[(c, t)] = "full"
                continue
            bt = np.ascontiguousarray(blk.T)
            key = bt.tobytes()
            if key not in pat_keys:
                pat_keys[key] = len(pats)
                # first tq column with any valid element: ops on this
                # pattern only touch [off, CHUNK)
                pat_off[len(pats)] = int(np.argmax(bt.any(axis=0)))
                pats.append(bt.astype(ml_dtypes.bfloat16))
            actions[(c, t)] = pat_keys[key]
    if not pats:
        pats.append(np.zeros((P, CHUNK), ml_dtypes.bfloat16))
    return actions, np.ascontiguousarray(np.stack(pats)), pat_off


def kernel(**inputs):
    global LAST_RESULT
    q_input = np.asarray(inputs["q_input"], dtype=np.float32)
    kv_input = np.asarray(inputs["kv_input"], dtype=np.float32)
    cos = np.asarray(inputs["cos"], dtype=np.float32)
    sin = np.asarray(inputs["sin"], dtype=np.float32)
    Wq = np.asarray(inputs["Wq"], dtype=np.float32)
    Wk = np.asarray(inputs["Wk"], dtype=np.float32)
    Wv = np.asarray(inputs["Wv"], dtype=np.float32)
    Wo = np.asarray(inputs["Wo"], dtype=np.float32)

    actions, pats, pat_off = _mask_actions(inputs["mask"])
    key = (tuple(sorted(actions.items())), pats.shape[0], tuple(sorted(pat_off.items())))
    if key not in _cache:
        _cache[key] = _build_program(actions, int(pats.shape[0]), pat_off)
    nc = _cache[key]

    bf = ml_dtypes.bfloat16
    cs2, ss2 = _rope_tables(cos, sin)

    def pack_x(x):
        # [T, D] -> [P, TCH, DT, CHUNK]: the exact SBUF x-chunk layout, so
        # each chunk DMA is one contiguous 16KB run per partition.
        return x.reshape(TCH, CHUNK, DT, P).transpose(3, 0, 2, 1).astype(bf)

    def pack_w(w):
        # [D, GD] -> [P, DT, GD]
        return w.reshape(DT, P, GD).transpose(1, 0, 2).astype(bf)

    def pack_wo(w):
        # [GD, D] -> [P, HPG, D]
        return w.reshape(HPG, P, D).transpose(1, 0, 2).astype(bf)

    xq = [pack_x(q_input[b]) for b in range(B)]
    xkv = [pack_x(kv_input[b]) for b in range(B)]
    wq_g = [pack_w(_interleave_heads(Wq[:, g * GD : (g + 1) * GD])) for g in range(GROUPS)]
    wk_g = [pack_w(_interleave_heads(Wk[:, g * GD : (g + 1) * GD])) for g in range(GROUPS)]
    wv_g = [pack_w(Wv[:, g * GD : (g + 1) * GD]) for g in range(GROUPS)]
    wo_g = [pack_wo(Wo[g * GD : (g + 1) * GD, :]) for g in range(GROUPS)]

    in_maps = []
    for core in range(NCORES):
        b, g = divmod(core, GROUPS)
        in_maps.append({
            "xqp": xq[b],
            "xkvp": xkv[b],
            "wq": wq_g[g],
            "wk": wk_g[g],
            "wv": wv_g[g],
            "wo": wo_g[g],
            "cs2": cs2,
            "ss2": ss2,
            "pat": pats,
        })

    from concourse import bass_utils

    res = bass_utils.run_bass_kernel_spmd(
        nc, in_maps, core_ids=list(range(NCORES)), trace=TRACE, tmpdir=TRACE_DIR
    )
    LAST_RESULT = res
    outs = [np.asarray(r["out"], dtype=np.float32) for r in res.results]
    full = np.stack(
        [sum(outs[b * GROUPS + g] for g in range(GROUPS)) for b in range(B)]
    )
    return np.ascontiguousarray(full.astype(np.float32))



# revision 27
# speedup vs baseline: 1.1220x; 1.0943x over previous
"""Multi-head causal attention with RoPE on 8 Trainium2 NeuronCores.

Sharding: data-parallel over batch (2 groups of 4 cores) x tensor-parallel
over heads (4 heads / 512 cols of Wq/Wk/Wv per core, 512 rows of Wo).
Each core computes its head-group's Q/K/V projections in transposed layout
([head_dim, seq] -- so no on-device transposes are ever needed), applies
RoPE, runs causal softmax attention (scores kept transposed [tk, tq];
row sums via a ones-vector matmul), and emits its partial output
projection.  The host sums the 4 partials per batch element.

Self-contained: shapes/sharding hardcoded for
  q_input/kv_input [2, 2048, 2048], 16 heads x 128 head_dim.
"""

import math

import numpy as np
import ml_dtypes

B, T, D, H = 2, 2048, 2048, 16
HD = 128          # head dim
HALF = HD // 2    # rope half
P = 128           # partitions
CHUNK = 512       # tq / free-dim chunk
NCORES = 8
GROUPS = 4        # head-groups (tensor-parallel degree per batch)
HPG = H // GROUPS # heads per group
GD = HPG * HD     # group width (512)
DT = D // P       # d-tiles (16)
TCH = T // CHUNK  # seq chunks (4)
TKT = T // P      # tk tiles (16)
CPT = CHUNK // P  # tk tiles per chunk (4)

TRACE = False       # set True before calling kernel() to capture an NTFF trace
TRACE_DIR = None    # optional fixed dir for trace artifacts
LAST_RESULT = None  # BassKernelResults of the last kernel() call
DEBUG_DUMPS = False # add QT/KT/V/AT debug outputs to the program

_cache = {}


def _build_program(actions, npat, pat_off):
    """Build the per-core Bass program (v2 schedule).

    actions: {(c, t): "full" | pattern_index} for every (tq-chunk, tk-tile)
    score block that has at least one unmasked element.

    v2 vs baseline: attention streams t-major across heads with the AV
    matmul LAGged 4 blocks behind its score matmul (so the scalar-engine
    exp never head-of-line-blocks the PE), the per-head row-sum matmuls
    are issued back-to-back into disjoint 32-col groups (concurrent on
    the PE sub-arrays), qproj(c+1)/oproj(c-1) matmul groups are drizzled
    into the attention stream as PE filler, evictions move off the
    scalar engine (DVE copies) during attention, the startup DMAs are
    split across the sync+scalar HWDGE queues, and the output DMA is
    batched per 128-row strip.
    """
    from contextlib import ExitStack

    import concourse.mybir as mybir
    import concourse.tile as tile
    from concourse import bacc
    from concourse.bass import ds, ts

    fp32 = mybir.dt.float32
    bf16 = mybir.dt.bfloat16
    Copy = mybir.ActivationFunctionType.Copy
    Exp = mybir.ActivationFunctionType.Exp
    SCALE = 1.0 / math.sqrt(HD)

    nc = bacc.Bacc(
        "TRN2",
        target_bir_lowering=False,
        debug=False,
        enable_asserts=False,
        num_devices=NCORES,
    )

    # All large inputs arrive HOST-PACKED in their exact SBUF layouts
    # (partition-major), so every DMA is one long contiguous run per
    # partition (16KB lines) instead of thousands of 1KB descriptors.
    xqp = nc.dram_tensor("xqp", [P, TCH, DT, CHUNK], bf16, kind="ExternalInput").ap()
    xkvp = nc.dram_tensor("xkvp", [P, TCH, DT, CHUNK], bf16, kind="ExternalInput").ap()
    wq = nc.dram_tensor("wq", [P, DT, GD], bf16, kind="ExternalInput").ap()
    wk = nc.dram_tensor("wk", [P, DT, GD], bf16, kind="ExternalInput").ap()
    wv = nc.dram_tensor("wv", [P, DT, GD], bf16, kind="ExternalInput").ap()
    wo = nc.dram_tensor("wo", [P, HPG, D], bf16, kind="ExternalInput").ap()
    # RoPE in head-dim-interleaved space (host permutes Wq/Wk columns so the
    # rope pair (j, j+64) lands on adjacent partitions (2j, 2j+1); scores are
    # invariant to a common Q/K head-dim permutation):
    #   rope'(x) = x * cs2 + swap_adjacent_pairs(x) * ss2
    # cs2[2j] = cs2[2j+1] = cos_j ; ss2[2j] = -sin_j, ss2[2j+1] = +sin_j
    cs2 = nc.dram_tensor("cs2", [P, T], bf16, kind="ExternalInput").ap()
    ss2 = nc.dram_tensor("ss2", [P, T], bf16, kind="ExternalInput").ap()
    pat = nc.dram_tensor("pat", [npat, P, CHUNK], bf16, kind="ExternalInput").ap()
    out = nc.dram_tensor("out", [T, D], bf16, kind="ExternalOutput").ap()

    with ExitStack() as ctx:
        tc = ctx.enter_context(tile.TileContext(nc))
        const_pool = ctx.enter_context(tc.tile_pool(name="const", bufs=1))
        xpool = ctx.enter_context(tc.tile_pool(name="xchunk", bufs=2))
        rope_pool = ctx.enter_context(tc.tile_pool(name="rope", bufs=2))
        exp_pool = ctx.enter_context(tc.tile_pool(name="exp", bufs=9))
        osb_pool = ctx.enter_context(tc.tile_pool(name="osb", bufs=2))
        lb_pool = ctx.enter_context(tc.tile_pool(name="lb", bufs=1))
        # PSUM: 8 banks total = a(2) + q(1) + o(4) + l(1).  qproj filler
        # accums get their own 1-buf pool: they are held open across
        # several filler units, and sharing a rotating slot with score
        # tiles would let a score matmul WAR-wait on PE work queued
        # behind it (head-of-line deadlock).
        a_psum = ctx.enter_context(tc.tile_pool(name="aps", bufs=2, space="PSUM"))
        q_psum = ctx.enter_context(tc.tile_pool(name="qps", bufs=1, space="PSUM"))
        o_psum = ctx.enter_context(tc.tile_pool(name="ops", bufs=4, space="PSUM"))
        l_psum = ctx.enter_context(tc.tile_pool(name="lps", bufs=1, space="PSUM"))

        # persistent SBUF tensors
        wq_sb = const_pool.tile([P, DT, GD], bf16, tag="wq")
        wk_sb = const_pool.tile([P, DT, GD], bf16, tag="wk")
        wv_sb = const_pool.tile([P, DT, GD], bf16, tag="wv")
        wo_sb = const_pool.tile([P, HPG, D], bf16, tag="wo")
        cs2_sb = const_pool.tile([P, T], bf16, tag="cs2")
        ss2_sb = const_pool.tile([P, T], bf16, tag="ss2")
        pat_sb = const_pool.tile([P, npat, CHUNK], bf16, tag="pat")
        ones_sb = const_pool.tile([P, 1], bf16, tag="ones")
        QT = const_pool.tile([P, HPG, T], bf16, tag="QT")
        KT = const_pool.tile([P, HPG, T], bf16, tag="KT")
        V = const_pool.tile([P, TKT, GD], bf16, tag="V")
        AT = const_pool.tile([P, HPG, T], bf16, tag="AT")

        # Startup: packed layouts make each dma_start ~128 descriptors, so
        # issue cost is negligible.  Piece sizes ramp [2,2,4,8] d-tiles:
        # the first K-proj matmuls start after ~0.5MB instead of 4MB.
        # cs2/ss2 ride the scalar HWDGE queue (idle at startup).
        # Transfer order matches need order: wk/xk d-pieces for the
        # d-major K ramp, wv split around the last piece (first V matmul
        # needs it ~10us after the last K d-tile).
        PIECES = [(0, 2), (2, 2), (4, 4), (8, 8)]
        xk0 = xpool.tile([P, DT, CHUNK], bf16, tag="xc")
        for s0, nd in PIECES[:3]:
            nc.sync.dma_start(wk_sb[:, s0:s0 + nd, :], wk[:, s0:s0 + nd, :])
            nc.sync.dma_start(xk0[:, s0:s0 + nd, :], xkvp[:, 0, s0:s0 + nd, :])
        nc.sync.dma_start(wv_sb[:, 0:8, :], wv[:, 0:8, :])
        for s0, nd in PIECES[3:]:
            nc.sync.dma_start(wk_sb[:, s0:s0 + nd, :], wk[:, s0:s0 + nd, :])
            nc.sync.dma_start(xk0[:, s0:s0 + nd, :], xkvp[:, 0, s0:s0 + nd, :])
        nc.sync.dma_start(wv_sb[:, 8:16, :], wv[:, 8:16, :])
        nc.scalar.dma_start(cs2_sb[:], cs2)
        nc.scalar.dma_start(ss2_sb[:], ss2)
        nc.vector.memset(ones_sb[:], 1.0)

        SHUF_MASK = [i + 1 - 2 * (i % 2) for i in range(32)]  # [1,0,3,2,...]
        IDENT32 = list(range(32))
        OCH = min(CHUNK, D)

        def rope_evict(ps, h, c, dest):
            # ps: PSUM [P, CHUNK] fp32, partitions = interleaved head_dim.
            # Raw copy always on ACT: a DVE fp32->bf16 copy lowers to a 1x
            # CAST (~690ns) so there is nothing to save on DVE, and the ACT
            # copy frees the PSUM slot through a much shallower queue.
            raw = rope_pool.tile([P, CHUNK], bf16, tag="raw")
            nc.scalar.activation(raw[:], ps[:], Copy)
            rsw = rope_pool.tile([P, CHUNK], bf16, tag="rsw")
            nc.vector.stream_shuffle(rsw[:], raw[:], SHUF_MASK)
            t1 = rope_pool.tile([P, CHUNK], bf16, tag="t1")
            nc.vector.tensor_mul(t1[:], raw[:], cs2_sb[:, ts(c, CHUNK)])
            t2 = rope_pool.tile([P, CHUNK], bf16, tag="t2")
            nc.vector.tensor_mul(t2[:], rsw[:], ss2_sb[:, ts(c, CHUNK)])
            nc.vector.tensor_add(dest[:, h, ts(c, CHUNK)], t1[:], t2[:])

        xq_tiles = {}

        def issue_xq(c):
            xq = xpool.tile([P, DT, CHUNK], bf16, tag="xc", name=f"xq{c}")
            nc.sync.dma_start(xq[:], xqp[:, c, :, :])
            xq_tiles[c] = xq

        def qproj_fillers(c, pool=None, tag="q"):
            # 16 filler units of 4 matmuls each (one PSUM accumulation per
            # head, rope-evicted at the end of its 4th unit).  Interleaved
            # fillers use the 1-buf q pool; qproj(0) standalone rotates
            # through the a pool.  The fillq order (all of one accum's
            # units before anything else touches the q pool) is what makes
            # the 1-buf sharing deadlock-free.
            units = []
            accum = {}
            if pool is None:
                pool = q_psum

            def unit(h, u):
                if u == 0:
                    accum[h] = pool.tile([P, CHUNK], fp32, tag=tag, name="qacc")
                ps = accum[h]
                for d in range(4 * u, 4 * u + 4):
                    nc.tensor.matmul(
                        ps[:], wq_sb[:, d, ts(h, HD)], xq_tiles[c][:, d, :],
                        start=(d == 0), stop=(d == DT - 1),
                    )
                if u == 3:
                    rope_evict(ps, h, c, QT)

            for h in range(HPG):
                for u in range(4):
                    units.append(lambda h=h, u=u: unit(h, u))
            return units

        # oproj: out[tq,:] partial rows, one (m, oc) unit = 4 matmuls +
        # evict into a [P, D] staging strip + one batched DMA per m.
        # In-attention units use the 1-buf q pool with a DVE evict (a
        # lagging eviction throttles only the filler stream, never the
        # scores); the tail rotates through q+a(x2) slots with evicts
        # alternating DVE/ACT so units retire back-to-back.
        obs_store = {}

        def emit_ounit(m, oc, pool, tag, evict_act):
            if oc == 0:
                obs_store[m] = osb_pool.tile([P, D], bf16, tag="ob", name="ob")
            ps = pool.tile([P, OCH], fp32, tag=tag, name="ops2")
            for h in range(HPG):
                nc.tensor.matmul(
                    ps[:], AT[:, h, ts(m, P)], wo_sb[:, h, ts(oc, OCH)],
                    start=(h == 0), stop=(h == HPG - 1),
                )
            if evict_act:
                nc.scalar.activation(obs_store[m][:, ts(oc, OCH)], ps[:], Copy)
            else:
                nc.vector.tensor_copy(obs_store[m][:, ts(oc, OCH)], ps[:])
            if oc == D // OCH - 1:
                nc.sync.dma_start(out[ts(m, P), :], obs_store[m][:])

        def oproj_descs(co):
            return [(m, oc) for m in range(4 * co, 4 * co + 4)
                    for oc in range(D // OCH)]

        def attn_chunk(c, qfill, ofill):
            # t-major across heads; AV lags its score matmul by LAG blocks
            # so the ACT exp sits between two PE ops that are never
            # adjacent in the PE queue.  Row-sum matmuls are emitted 4
            # back-to-back into col-groups {0,32,64,96} of one PSUM bank
            # (concurrent sub-array execution), one group per t directly
            # on the es tiles (no DVE pre-summing).  qproj fillers pop at
            # a uniform rate; oproj units pop at most every OSPACING
            # blocks (their single PSUM slot + eviction latency caps the
            # sustainable cadence at ~2.6us).
            tlist = [t for t in range(TKT) if (c, t) in actions]
            full_ts = [t for t in tlist if actions[(c, t)] == "full"]
            diag_ts = [t for t in tlist if actions[(c, t)] != "full"]
            order_ts = full_ts + diag_ts
            n_lgrp = len(tlist)
            blocks = [(t, h) for t in order_ts for h in range(HPG)]
            nb = len(blocks)
            LAG = 5

            lpst = l_psum.tile([P, CHUNK], fp32, tag="l")
            opst = [o_psum.tile([P, CHUNK], fp32, tag="o", name=f"o{h}")
                    for h in range(HPG)]
            es_store = {}
            lgrp = [0]
            av_cnt = [0] * HPG
            nav = len(tlist)

            def emit_s(t, h):
                a = actions[(c, t)]
                off = 0 if a == "full" else pat_off[a]
                w = CHUNK - off
                sp = a_psum.tile([P, CHUNK], fp32, tag="a", name="sps")
                nc.tensor.matmul(
                    sp[:, ds(off, w)], KT[:, h, ts(t, P)],
                    QT[:, h, ds(c * CHUNK + off, w)],
                    start=True, stop=True,
                )
                es = exp_pool.tile([P, CHUNK], bf16, tag="es")
                nc.scalar.activation(es[:, ds(off, w)], sp[:, ds(off, w)],
                                     Exp, scale=SCALE)
                if a != "full":
                    nc.vector.tensor_mul(
                        es[:, ds(off, w)], es[:, ds(off, w)],
                        pat_sb[:, a, ds(off, w)],
                    )
                es_store[(t, h)] = (es, off, w)

            def lmm(hh, src, off, w):
                r = 32 * hh
                nc.tensor.matmul(
                    lpst[r:r + 1, ds(off, w)], ones_sb[:], src[:, ds(off, w)],
                    start=(lgrp[0] == 0), stop=(lgrp[0] == n_lgrp - 1),
                    tile_position=(0, r),
                )

            def emit_av(t, h):
                es, off, w = es_store[(t, h)]
                nc.tensor.matmul(
                    opst[h][:, ds(off, w)], V[:, t, ts(h, HD)], es[:, ds(off, w)],
                    start=(av_cnt[h] == 0), stop=(av_cnt[h] == nav - 1),
                )
                av_cnt[h] += 1
                if h == HPG - 1:
                    for hh in range(HPG):
                        ess, off2, w2 = es_store[(t, hh)]
                        lmm(hh, ess, off2, w2)
                    lgrp[0] += 1

            OSPACING = 4
            fill_acc = 0.0
            fill_rate = len(qfill) / nb if nb else 0.0
            for i in range(nb + LAG):
                if i >= LAG:
                    emit_av(*blocks[i - LAG])
                fill_acc += fill_rate
                while fill_acc >= 1.0 and qfill:
                    qfill.pop(0)()
                    fill_acc -= 1.0
                # o-units share the q slot with qproj accums: only pop
                # when no accum is open (qfill length is a multiple of 4)
                if (i >= LAG and i % OSPACING == 0 and ofill
                        and len(qfill) % 4 == 0):
                    m, oc = ofill.pop(0)
                    emit_ounit(m, oc, q_psum, "q", evict_act=False)
                if i < nb:
                    emit_s(*blocks[i])

            # unnormalized O^T eviction (ACT: o-banks free through a
            # shallow queue at the chunk boundary), then the 1/l chain
            for h in range(HPG):
                nc.scalar.activation(AT[:, h, ts(c, CHUNK)], opst[h][:], Copy)
            rec = lb_pool.tile([P, CHUNK], fp32, tag="rec")
            nc.vector.reciprocal_approx_fast(rec[:], lpst[:])
            for h in range(HPG):
                r = 32 * h
                if h == 0:
                    src = rec
                else:
                    src = lb_pool.tile([32, CHUNK], fp32, tag="shf")
                    nc.vector.stream_shuffle(src[:], rec[r:r + 32, :], IDENT32)
                lbs = lb_pool.tile([P, CHUNK], fp32, tag="lbs")
                nc.gpsimd.partition_broadcast(lbs[:], src[0:1, :])
                nc.vector.tensor_mul(
                    AT[:, h, ts(c, CHUNK)], AT[:, h, ts(c, CHUNK)], lbs[:]
                )

        # ---- K^T / V projections over all chunks ----
        # K accums run d-MAJOR with all four heads' PSUM banks open (the
        # o pool is idle during this phase): each arriving wk/xk d-piece
        # is consumed immediately, so the startup ramp tracks the DMA.
        xk_tiles = {0: xk0}
        for c in range(TCH):
            xk = xk_tiles[c]
            if c + 1 < TCH:
                nxt = xpool.tile([P, DT, CHUNK], bf16, tag="xc", name=f"xk{c + 1}")
                nc.sync.dma_start(nxt[:], xkvp[:, c + 1, :, :])
                xk_tiles[c + 1] = nxt
            kps = [o_psum.tile([P, CHUNK], fp32, tag="o", name=f"kps{h}")
                   for h in range(HPG)]
            for d in range(DT):
                for h in range(HPG):
                    nc.tensor.matmul(
                        kps[h][:], wk_sb[:, d, ts(h, HD)], xk[:, d, :],
                        start=(d == 0), stop=(d == DT - 1),
                    )
            for h in range(HPG):
                rope_evict(kps[h], h, c, KT)
            for s in range(CPT):
                ps = a_psum.tile([P, GD], fp32, tag="a", name="vps")
                for d in range(DT):
                    nc.tensor.matmul(
                        ps[:], xk[:, d, ts(s, P)], wv_sb[:, d, :],
                        start=(d == 0), stop=(d == DT - 1),
                    )
                nc.scalar.activation(V[:, c * CPT + s, :], ps[:], Copy)
            if c == 0:
                nc.sync.dma_start(wq_sb[:], wq[:])
                nc.sync.dma_start(pat_sb[:], pat.rearrange("j p n -> p j n"))
            elif c == 1:
                nc.sync.dma_start(wo_sb[:], wo[:])
            elif c == 2:
                issue_xq(3)
            elif c == 3:
                issue_xq(2)

        # ---- attention, LAST chunk FIRST ----
        # attn(3) is the one ACT-bound chunk (64 exps pace it); giving it
        # the qproj(2) fillers -- whose cadence is not eviction-limited --
        # fills its PE idle.  oproj drips into the attention stream at
        # OSPACING and the remainder runs in a PE-dense tail.
        for u in qproj_fillers(3, pool=a_psum, tag="a"):
            u()
        ORDER = [3, 2, 1, 0]
        qfill, ofill = [], []
        for idx, c in enumerate(ORDER):
            if c == 3:
                issue_xq(1)
            elif c == 2:
                issue_xq(0)
            if c - 1 >= 0:
                qfill.extend(qproj_fillers(c - 1))
            if idx > 0:
                ofill.extend(oproj_descs(ORDER[idx - 1]))
            attn_chunk(c, qfill, ofill)
        for u in qfill:
            u()
        ofill.extend(oproj_descs(ORDER[-1]))
        tail_pools = [(q_psum, "q"), (a_psum, "a"), (a_psum, "a")]
        k = 0
        while ofill:
            m, oc = ofill.pop(0)
            pool, tag = tail_pools[k % 3]
            emit_ounit(m, oc, pool, tag, evict_act=(k % 2 == 1))
            k += 1

        if DEBUG_DUMPS:
            for nm, sb in [("dQT", QT), ("dKT", KT), ("dV", V), ("dAT", AT)]:
                dt_ = nc.dram_tensor(nm, list(sb.shape), bf16, kind="ExternalOutput").ap()
                nc.sync.dma_start(dt_[:], sb[:])

    nc.compile()
    return nc


def _interleave_heads(W):
    """Permute each 128-wide head block of columns: new[2j]=old[j], new[2j+1]=old[64+j]."""
    d, gd = W.shape
    return np.ascontiguousarray(
        W.reshape(d, gd // HD, 2, HALF).transpose(0, 1, 3, 2).reshape(d, gd)
    )


def _rope_tables(cos, sin):
    """cs2[2j]=cs2[2j+1]=cos_j ; ss2[2j]=-sin_j, ss2[2j+1]=+sin_j  (both [128, T])."""
    bf = ml_dtypes.bfloat16
    cosT = np.ascontiguousarray(cos.T)  # [HALF, T]
    sinT = np.ascontiguousarray(sin.T)
    cs2 = np.repeat(cosT, 2, axis=0).astype(bf)
    ss2 = np.stack([-sinT, sinT], axis=1).reshape(HD, -1).astype(bf)
    return cs2, ss2


def _mask_actions(mask):
    """Classify every [CHUNK tq x P tk] score block of the mask.

    Returns ({(c, t): "full" | pattern_idx}, patterns [npat, P, CHUNK] bf16).
    Blocks with no unmasked element are omitted (skipped entirely).
    Patterns are stored transposed ([tk, tq]) to match the score layout.
    """
    m = np.asarray(mask).reshape(T, T).astype(bool)
    actions = {}
    pats = []
    pat_keys = {}
    pat_off = {}
    for c in range(TCH):
        for t in range(TKT):
            blk = m[c * CHUNK : (c + 1) * CHUNK, t * P : (t + 1) * P]
            if not blk.any():
                continue
            if blk.all():
                actions[(c, t)] = "full"
                continue
            bt = np.ascontiguousarray(blk.T)
            key = bt.tobytes()
            if key not in pat_keys:
                pat_keys[key] = len(pats)
                # first tq column with any valid element: ops on this
                # pattern only touch [off, CHUNK)
                pat_off[len(pats)] = int(np.argmax(bt.any(axis=0)))
                pats.append(bt.astype(ml_dtypes.bfloat16))
            actions[(c, t)] = pat_keys[key]
    if not pats:
        pats.append(np.zeros((P, CHUNK), ml_dtypes.bfloat16))
    return actions, np.ascontiguousarray(np.stack(pats)), pat_off


def kernel(**inputs):
    global LAST_RESULT
    q_input = np.asarray(inputs["q_input"], dtype=np.float32)
    kv_input = np.asarray(inputs["kv_input"], dtype=np.float32)
    cos = np.asarray(inputs["cos"], dtype=np.float32)
    sin = np.asarray(inputs["sin"], dtype=np.float32)
    Wq = np.asarray(inputs["Wq"], dtype=np.float32)
    Wk = np.asarray(inputs["Wk"], dtype=np.float32)
    Wv = np.asarray(inputs["Wv"], dtype=np.float32)
    Wo = np.asarray(inputs["Wo"], dtype=np.float32)

    actions, pats, pat_off = _mask_actions(inputs["mask"])
    key = (tuple(sorted(actions.items())), pats.shape[0], tuple(sorted(pat_off.items())))
    if key not in _cache:
        _cache[key] = _build_program(actions, int(pats.shape[0]), pat_off)
    nc = _cache[key]

    bf = ml_dtypes.bfloat16
    cs2, ss2 = _rope_tables(cos, sin)

    def pack_x(x):
        # [T, D] -> [P, TCH, DT, CHUNK]: the exact SBUF x-chunk layout, so
        # each chunk DMA is one contiguous 16KB run per partition.
        return x.reshape(TCH, CHUNK, DT, P).transpose(3, 0, 2, 1).astype(bf)

    def pack_w(w):
        # [D, GD] -> [P, DT, GD]
        return w.reshape(DT, P, GD).transpose(1, 0, 2).astype(bf)

    def pack_wo(w):
        # [GD, D] -> [P, HPG, D]
        return w.reshape(HPG, P, D).transpose(1, 0, 2).astype(bf)

    xq = [pack_x(q_input[b]) for b in range(B)]
    xkv = [pack_x(kv_input[b]) for b in range(B)]
    wq_g = [pack_w(_interleave_heads(Wq[:, g * GD : (g + 1) * GD])) for g in range(GROUPS)]
    wk_g = [pack_w(_interleave_heads(Wk[:, g * GD : (g + 1) * GD])) for g in range(GROUPS)]
    wv_g = [pack_w(Wv[:, g * GD : (g + 1) * GD]) for g in range(GROUPS)]
    wo_g = [pack_wo(Wo[g * GD : (g + 1) * GD, :]) for g in range(GROUPS)]

    in_maps = []
    for core in range(NCORES):
        b, g = divmod(core, GROUPS)
        in_maps.append({
            "xqp": xq[b],
            "xkvp": xkv[b],
            "wq": wq_g[g],
            "wk": wk_g[g],
            "wv": wv_g[g],
            "wo": wo_g[g],
            "cs2": cs2,
            "ss2": ss2,
            "pat": pats,
        })

    from concourse import bass_utils

    res = bass_utils.run_bass_kernel_spmd(
        nc, in_maps, core_ids=list(range(NCORES)), trace=TRACE, tmpdir=TRACE_DIR
    )
    LAST_RESULT = res
    outs = [np.asarray(r["out"], dtype=np.float32) for r in res.results]
    full = np.stack(
        [sum(outs[b * GROUPS + g] for g in range(GROUPS)) for b in range(B)]
    )
    return np.ascontiguousarray(full.astype(np.float32))

